# revision 28
# baseline (speedup 1.0000x reference)
"""Biquad peaking-EQ IIR filter on 8 Trainium2 NeuronCores.

Math: the reference applies a 2nd-order IIR (biquad) along time for each of
the 64 independent signals (32 batch x 2 channels, T=524288).  The filter's
poles have magnitude sqrt(a2) ~ 0.919, so the impulse response decays below
1e-10 (relative, L2) after 256 samples.  We therefore compute the zero-state
response as a truncated-FIR convolution (256 taps), which is embarrassingly
parallel:

    y[n] = sum_k h[k] x[n-k]        (x[<0] = 0)

Blocked formulation on the 128x128 tensor engine, f16 end to end:

  * The HOST pre/post-formats the data (numpy, not on the device critical
    path): x is cast to f16 and transposed per signal into the block-major
    view X'[j, c] = x[128c + j] (with one zeroed halo column c = -1), and
    the block-major result Y'[g, c] = y[128c + g] is transposed back after
    the run.  The device therefore only ever does large contiguous DMAs
    (4-8 KiB per partition line) - no on-device transposes at all.
  * On-device conv: two Toeplitz matrices are the stationary operands,
    T0[j, g] = h[g-j] (g >= j, taps 0..127) and T1[j, g] = h[128+g-j]
    (taps 128..255).  For each 512-column chunk of X', two PSUM-accumulated
    matmuls compute

        Y'[g, c] = sum_j T0[j, g] X'[j, c] + sum_j T1[j, g] X'[j, c-1]

    PSUM (fp32) is evacuated to f16 SBUF by the Vector and Scalar engines
    (split between them to balance), and stored contiguously.

f16 quantization of x/h/y adds ~3e-4 relative L2 error, far below the 2e-2
gate, and halves all HBM traffic vs fp32.  All FLOPs stay on the PE.

Sharding: pure data parallel - 64 signals / 8 cores = 8 signals per core.

Scheduling note: every TPB 64-byte instruction has a single semaphore-wait
slot, but Tile's slot-release deps routinely put 2+ waits on one
instruction (walrus then fails with "Too many sync wait commands").
_strip_redundant_waits post-processes the scheduled BIR: it computes
transitive completion guarantees (engine queues are in-order FIFO; an
instruction completes only after its waits held; a semaphore's v-th update
implies its earlier ones) and (a) drops waits provably implied by another
wait on the same instruction, (b) splits any remaining multi-wait set into
single-wait NoOps ahead of the instruction on the same queue.  The patched
BIR is returned via an instance-level to_json_bytes override that
bass2jax's lowering picks up.
"""

import math

import numpy as np

SAMPLE_RATE = 44100.0

# Problem geometry (hardcoded per harness contract).
B_FULL, C_FULL, T_FULL = 32, 2, 524288
N_CORES = 8
SIGS_PER_CORE = (B_FULL * C_FULL) // N_CORES  # 8
L = 128             # block size == PE array dim
NBLK = T_FULL // L   # 4096 block columns per signal
HALVES = 2           # pipeline units per signal (finer overlap)
HBLK = NBLK // HALVES  # block columns per half-signal
QW = 512             # matmul chunk width (1 PSUM bank of fp32)
NQ = HBLK // QW      # 4 chunks per half-signal
XP_ROW = 2176        # padded input row (halo + HBLK = 2049, 256B-aligned)


def _filter_coeffs(center_freq: float, q: float, gain: float):
    """torchaudio equalizer_biquad coefficients, normalized by a0 (float64)."""
    g = min(max(gain, 0.1), 10.0)
    w0 = 2.0 * math.pi * center_freq / SAMPLE_RATE
    A = math.exp(g / 40.0 * math.log(10.0))
    alpha = math.sin(w0) / (2.0 * q)
    b0 = 1.0 + alpha * A
    b1 = -2.0 * math.cos(w0)
    b2 = 1.0 - alpha * A
    a0 = 1.0 + alpha / A
    a1 = b1
    a2 = 1.0 - alpha / A
    return b0 / a0, b1 / a0, b2 / a0, a1 / a0, a2 / a0


def _impulse_response(center_freq: float, q: float, gain: float, n: int = 256):
    b0, b1, b2, a1, a2 = _filter_coeffs(center_freq, q, gain)
    h = np.zeros(n, dtype=np.float64)
    x1 = x2 = y1 = y2 = 0.0
    for i in range(n):
        xn = 1.0 if i == 0 else 0.0
        yn = b0 * xn + b1 * x1 + b2 * x2 - a1 * y1 - a2 * y2
        x2, x1 = x1, xn
        y2, y1 = y1, yn
        h[i] = yn
    return h


def _toeplitz_mats(h: np.ndarray):
    """T0[j,g] = h[g-j] (g>=j else 0); T1[j,g] = h[128+g-j].  Stationary
    matmul operands (lhsT): out = lhsT.T @ rhs."""
    j = np.arange(L)[:, None]
    g = np.arange(L)[None, :]
    d0 = g - j
    t0 = np.where(d0 >= 0, h[np.clip(d0, 0, len(h) - 1)], 0.0)
    d1 = 128 + g - j
    t1 = h[np.clip(d1, 0, len(h) - 1)]
    return t0.astype(np.float16), t1.astype(np.float16)


_NC_CACHE = {}


def _build_nc(n_sigs: int = SIGS_PER_CORE):
    """Build the per-core Bass program (same NEFF on all cores)."""
    import concourse.bass as bass
    import concourse.mybir as mybir
    import concourse.tile as tile

    f16 = mybir.dt.float16
    f32 = mybir.dt.float32
    nc = bass.Bass("TRN2")

    # Block-major input, one row per half-signal, with a halo column:
    # xp[u, j, 0] = X'[j, -1] of the half (zero for the first half, the
    # last column of the previous half otherwise), xp[u, j, 1+c] = X' col c.
    # Output block-major: yb[u, g, c].
    # Row stride padded to a multiple of 256B (2176 cols = 4352B) so every
    # partition line starts DRAM-page-aligned (dram-page-size=256).
    n_units = n_sigs * HALVES
    xp = nc.dram_tensor("xp", [n_units, L, XP_ROW], f16, kind="ExternalInput")
    t0d = nc.dram_tensor("t0", [L, L], f16, kind="ExternalInput")
    t1d = nc.dram_tensor("t1", [L, L], f16, kind="ExternalInput")
    yb = nc.dram_tensor("yb", [n_units, L, HBLK], f16, kind="ExternalOutput")

    with tile.TileContext(nc) as tc:
        with (
            tc.tile_pool(name="consts", bufs=1) as consts,
            tc.tile_pool(name="xs", bufs=8) as xs_pool,
            tc.tile_pool(name="yo", bufs=4) as yo_pool,
            tc.tile_pool(name="mm_ps", bufs=2, space="PSUM") as mm_ps,
        ):
            # Consts go over the (initially idle) store queue so the first
            # unit's load is not queued behind them.
            t0s = consts.tile([L, L], f16)
            t1s = consts.tile([L, L], f16)
            nc.scalar.dma_start(t0s[:], t0d[:])
            nc.scalar.dma_start(t1s[:], t1d[:])

            for u in range(n_units):
                # Loads dispatch from SP, stores from ACT: mixing directions
                # on one engine queue head-of-line-blocks later loads behind
                # a store dispatch that waits on compute.
                # ---- contiguous load (f16, 4KiB per partition line) ----
                xs = xs_pool.tile([L, HBLK + 1], f16)
                nc.sync.dma_start(xs[:], xp[u][:, : HBLK + 1])

                # ---- conv: all-T0 pass, then all-T1 pass; 4 PSUM banks per
                # unit, double-buffered across units ----
                pps = [
                    mm_ps.tile([L, QW], f32, tag=f"mm{q}", name=f"pp{q}")
                    for q in range(NQ)
                ]
                for q in range(NQ):
                    nc.tensor.matmul(
                        pps[q][:], t0s[:], xs[:, 1 + QW * q : 1 + QW * (q + 1)],
                        start=True, stop=False,
                    )
                for q in range(NQ):
                    nc.tensor.matmul(
                        pps[q][:], t1s[:], xs[:, QW * q : QW * (q + 1)],
                        start=False, stop=True,
                    )

                # ---- evac PSUM -> f16 (DVE 3 / ACT 1 per unit; GpSimd
                # cannot read PSUM), then store the unit ----
                yo = yo_pool.tile([L, HBLK], f16)
                evac_eng = [0, 1, 0, 0]  # 0=DVE, 1=ACT
                for q in range(NQ):
                    dst = yo[:, QW * q : QW * (q + 1)]
                    if evac_eng[q]:
                        nc.scalar.copy(dst, pps[q][:])
                    else:
                        nc.vector.tensor_copy(dst, pps[q][:])
                nc.scalar.dma_start(yb[u][:], yo[:])

    return nc


def _dedup_ldweights(bir_bytes: bytes) -> bytes:
    """Turn redundant PE Ldweights into NoOps.

    The emission interleaves `ld T0, mm, ld T0, mm, ...` within each all-T0
    (then all-T1) pass, but PE weights stay loaded across Matmults, so a
    Ldweights whose operand signature matches the previous one (with only
    Matmult/NoOp in between on the PE queue) is a pure reload.  walrus's
    ldw-opt would drop these but is compiled out (--enable-ldw-opt=false),
    so neutralize them here: opcode -> NoOp, operands cleared, sync_info
    (waits + semaphore updates) preserved so the schedule is unchanged."""
    import json

    bir = json.loads(bir_bytes)

    def signature(i):
        sig = {k: v for k, v in i.items()
               if k not in ("name", "sync_info", "debug")}
        return json.dumps(sig, sort_keys=True)

    n_dropped = 0

    def walk(block):
        nonlocal n_dropped
        last_sig = None
        for i in block.get("instructions", []):
            if i.get("engine") != "PE":
                continue
            op = i.get("opcode")
            if op == "Ldweights":
                sig = signature(i)
                if sig == last_sig:
                    keep = {"debug", "engine", "ins", "name", "opcode",
                            "outs", "sync_info"}
                    for k in [k for k in i if k not in keep]:
                        del i[k]
                    i["opcode"] = "NoOp"
                    i["ins"] = []
                    i["outs"] = []
                    n_dropped += 1
                else:
                    last_sig = sig
            elif op not in ("Matmult", "NoOp"):
                last_sig = None
        for sub in block.get("blocks", []):
            walk(sub)

    for b in bir["functions"][0]["blocks"]:
        walk(b)
    return json.dumps(bir).encode()


def _strip_redundant_waits(bir_bytes: bytes) -> bytes:
    """PE Matmult/Ldweights lower to TPB instructions with a single
    semaphore-wait slot, but Tile's slot-release deps put 2 waits (old-writer
    PE completion + old-reader DVE completion) on the first toucher of every
    reused PSUM slot.  The PE wait is transitively implied: the DVE evac copy
    whose completion the instruction also waits on had itself waited on those
    PE completions.  Prove the implication with a completion-guarantee
    dataflow (rules: an instruction completes only after its waits hold; TPB
    engine queues are in-order FIFO; a semaphore's v-th update implies its
    earlier updates) and drop provably-redundant waits; raise if a >1-wait
    matmul can't be reduced."""
    import json

    bir = json.loads(bir_bytes)
    insts = []
    containers = []  # (list, index) for each inst, for NoOp insertion

    def walk(block):
        lst = block.get("instructions", [])
        for idx, i in enumerate(lst):
            insts.append(i)
            containers.append((lst, idx))
        for sub in block.get("blocks", []):
            walk(sub)

    for b in bir["functions"][0]["blocks"]:
        walk(b)

    # Per-sem update timeline: list of (cumulative_value, inst_idx).
    timelines = {}
    for k, i in enumerate(insts):
        for u in i.get("sync_info", {}).get("on_update", []) or []:
            if u.get("sync_type") != "semaphore":
                continue
            tl = timelines.setdefault(u["ant_name"], [])
            prev = tl[-1][0] if tl else 0
            tl.append((prev + int(u.get("update_value", 1)), k))

    def producer(sem, val):
        """Index of the instruction whose update first brings sem >= val."""
        tl = timelines.get(sem)
        if not tl:
            return None
        import bisect
        pos = bisect.bisect_left(tl, (val, -1))
        if pos == len(tl):
            return None
        return tl[pos][1]

    IN_ORDER_ENGINES = {"PE", "DVE", "Activation", "Pool", "SP"}
    NOT_IN_ORDER_OPCODES = {"DMACopy", "DmaTransposeAnt"}  # complete on DMA queues

    # guarantees[k]: sem -> max value known to hold when inst k completes.
    guarantees = [dict() for _ in insts]
    prev_by_engine = {}
    preds = []  # per-inst: (same-engine pred, own waits, own updates)
    for k, i in enumerate(insts):
        eng = i.get("engine")
        in_order = eng in IN_ORDER_ENGINES and i.get("opcode") not in NOT_IN_ORDER_OPCODES
        pred = prev_by_engine.get(eng) if in_order else None
        preds.append(pred)
        if in_order:
            prev_by_engine[eng] = k

    def merge(dst, src):
        changed = False
        for s, v in src.items():
            if dst.get(s, 0) < v:
                dst[s] = v
                changed = True
        return changed

    for _pass in range(3):
        changed = False
        for k, i in enumerate(insts):
            g = guarantees[k]
            si = i.get("sync_info", {})
            for w in si.get("on_wait", []) or []:
                if w.get("sync_type") != "semaphore":
                    continue
                v = int(w["wait_value"])
                if g.get(w["ant_name"], 0) < v:
                    g[w["ant_name"]] = v
                    changed = True
                p = producer(w["ant_name"], v)
                if p is not None:
                    changed |= merge(g, guarantees[p])
            if preds[k] is not None:
                changed |= merge(g, guarantees[preds[k]])
        # Own updates fire at completion; same-sem update chains are FIFO
        # (engine queue or DMA queue), so the v-th updater inherits the
        # (v-1)-th updater's guarantees.
        for sem, tl in timelines.items():
            prev_idx = None
            for cum, k in tl:
                if guarantees[k].get(sem, 0) < cum:
                    guarantees[k][sem] = cum
                    changed = True
                if prev_idx is not None:
                    changed |= merge(guarantees[k], guarantees[prev_idx])
                prev_idx = k
        if not changed:
            break

    STRIP_OPCODES = {
        "Matmult", "Ldweights", "TensorCopy", "Memset", "DMACopy",
        "DmaTransposeAnt", "Activation", "TensorScalarAffineSelect",
        "TensorTensor", "TensorScalarPtr", "TensorReduce", "Drain", "NoOp",
    }
    stripped = 0
    inserts = []  # (list, index, [noop dicts])
    for k, i in enumerate(insts):
        if i.get("opcode") not in STRIP_OPCODES:
            continue
        si = i.get("sync_info", {})
        waits = si.get("on_wait", []) or []
        if len(waits) <= 1:
            continue
        # Drop every wait implied by another (not-yet-dropped) wait's
        # producer guarantee.
        kept = list(waits)
        changed = True
        while changed:
            changed = False
            for w in list(kept):
                if len(kept) == 1:
                    break
                for w2 in kept:
                    if w2 is w:
                        continue
                    p = producer(w2["ant_name"], int(w2["wait_value"]))
                    if p is not None and guarantees[p].get(w["ant_name"], 0) >= int(
                        w["wait_value"]
                    ):
                        kept.remove(w)
                        changed = True
                        break
        stripped += len(waits) - len(kept)
        si["on_wait"] = [kept[-1]]
        if len(kept) > 1:
            # Split remaining waits onto single-wait NoOps ahead of the
            # instruction on the same engine queue.
            lst, idx = containers[k]
            noops = [
                {
                    "debug": i.get("debug", 0),
                    "engine": i.get("engine"),
                    "ins": [],
                    "name": f"{i['name']}-w{j}",
                    "opcode": "NoOp",
                    "outs": [],
                    "sync_info": {"on_wait": [w], "on_update": []},
                }
                for j, w in enumerate(kept[:-1])
            ]
            inserts.append((lst, idx, noops))

    # Apply insertions (descending index per list keeps positions valid).
    from collections import defaultdict
    by_list = defaultdict(list)
    for lst, idx, noops in inserts:
        by_list[id(lst)].append((lst, idx, noops))
    for entries in by_list.values():
        for lst, idx, noops in sorted(entries, key=lambda e: -e[1]):
            lst[idx:idx] = noops

    out = json.dumps(bir).encode()
    return out


def audit_waits(bir_bytes):
    """Flag instructions with more than the single hardware wait slot."""
    import json

    bir = json.loads(bir_bytes)
    checked = {
        "Matmult", "Ldweights", "TensorCopy", "Memset", "DMACopy",
        "DmaTransposeAnt", "Activation", "TensorScalarAffineSelect",
        "TensorTensor", "TensorScalarPtr", "TensorReduce",
    }
    bad = []
    def walk(block):
        for i in block.get("instructions", []):
            if i.get("opcode") not in checked:
                continue
            w = i.get("sync_info", {}).get("on_wait", [])
            if len(w) > 1:
                bad.append((i["name"], i.get("opcode"), i.get("engine"),
                            [(x["ant_name"], x["wait_value"]) for x in w]))
        for sub in block.get("blocks", []):
            walk(sub)
    for b in bir["functions"][0]["blocks"]:
        walk(b)
    return bad


def _get_nc(n_sigs: int = SIGS_PER_CORE):
    if n_sigs not in _NC_CACHE:
        nc = _build_nc(n_sigs)
        patched = _strip_redundant_waits(
            _dedup_ldweights(type(nc).to_json_bytes(nc))
        )
        bad = audit_waits(patched)
        if bad:
            raise RuntimeError(f"multi-wait instructions remain: {bad[:5]}")
        nc.to_json_bytes = lambda: patched
        _NC_CACHE[n_sigs] = nc
    return _NC_CACHE[n_sigs]


def _to_blockmajor(x64: np.ndarray) -> np.ndarray:
    """[64, T] f16 -> [64*HALVES, 128, HBLK+1] f16 half-signal units.

    Unit (s, h) holds X'[s][:, 2048h : 2048(h+1)] in cols 1.., with col 0
    the halo (zero for h=0, else the last column of the previous half)."""
    n = x64.shape[0]
    xb = x64.reshape(n, NBLK, L).transpose(0, 2, 1)  # [n, 128, NBLK]
    xp = np.zeros((n, HALVES, L, XP_ROW), dtype=np.float16)
    for h_ in range(HALVES):
        xp[:, h_, :, 1 : HBLK + 1] = xb[:, :, HBLK * h_ : HBLK * (h_ + 1)]
        if h_ > 0:
            xp[:, h_, :, 0] = xb[:, :, HBLK * h_ - 1]
    return xp.reshape(n * HALVES, L, XP_ROW)


def _from_blockmajor(yb: np.ndarray) -> np.ndarray:
    """[64*HALVES, 128, HBLK] f16 -> [64, T] f16."""
    n = yb.shape[0] // HALVES
    yb = yb.reshape(n, HALVES, L, HBLK).transpose(0, 1, 3, 2)  # [n,H,HBLK,128]
    return np.ascontiguousarray(yb).reshape(n, T_FULL)


def run_spmd(x64: np.ndarray, t0: np.ndarray, t1: np.ndarray, trace: bool = False):
    """x64: [64, T] float16 -> [64, T] float16 (plus BassKernelResults)."""
    from concourse.bass_utils import run_bass_kernel_spmd

    nc = _get_nc()
    xp = _to_blockmajor(x64)
    upc = SIGS_PER_CORE * HALVES  # units per core
    in_maps = [
        {
            "xp": np.ascontiguousarray(xp[upc * c : upc * (c + 1)]),
            "t0": t0,
            "t1": t1,
        }
        for c in range(N_CORES)
    ]
    res = run_bass_kernel_spmd(
        nc, in_maps, core_ids=list(range(N_CORES)), trace=trace
    )
    yb = np.concatenate([res.results[c]["yb"] for c in range(N_CORES)], axis=0)
    return _from_blockmajor(yb), res


def kernel(x, center_freq, q, gain, t=0, **_unused):
    x = np.asarray(x)
    assert x.shape == (B_FULL, C_FULL, T_FULL), x.shape
    cf = float(np.asarray(center_freq).reshape(-1)[0])
    qv = float(np.asarray(q).reshape(-1)[0])
    gv = float(np.asarray(gain).reshape(-1)[0])

    h = _impulse_response(cf, qv, gv)
    t0, t1 = _toeplitz_mats(h)

    x64 = np.ascontiguousarray(
        x.reshape(B_FULL * C_FULL, T_FULL), dtype=np.float16
    )
    out, _ = run_spmd(x64, t0, t1, trace=False)
    return out.reshape(B_FULL, C_FULL, T_FULL).astype(np.float32)


# revision 29
# speedup vs baseline: 1.0316x; 1.0316x over previous
"""Biquad peaking-EQ IIR filter on 8 Trainium2 NeuronCores.

Math: the reference applies a 2nd-order IIR (biquad) along time for each of
the 64 independent signals (32 batch x 2 channels, T=524288).  The filter's
poles have magnitude sqrt(a2) ~ 0.919, so the impulse response decays below
1e-10 (relative, L2) after 256 samples.  We therefore compute the zero-state
response as a truncated-FIR convolution (256 taps), which is embarrassingly
parallel:

    y[n] = sum_k h[k] x[n-k]        (x[<0] = 0)

Blocked formulation on the 128x128 tensor engine, f16 end to end:

  * The HOST pre/post-formats the data (numpy, not on the device critical
    path): x is cast to f16 and transposed per signal into the block-major
    view X'[j, c] = x[128c + j] (with one zeroed halo column c = -1), and
    the block-major result Y'[g, c] = y[128c + g] is transposed back after
    the run.  The device therefore only ever does large contiguous DMAs
    (4-8 KiB per partition line) - no on-device transposes at all.
  * On-device conv: two Toeplitz matrices are the stationary operands,
    T0[j, g] = h[g-j] (g >= j, taps 0..127) and T1[j, g] = h[128+g-j]
    (taps 128..255).  For each 512-column chunk of X', two PSUM-accumulated
    matmuls compute

        Y'[g, c] = sum_j T0[j, g] X'[j, c] + sum_j T1[j, g] X'[j, c-1]

    PSUM (fp32) is evacuated to f16 SBUF by the Vector and Scalar engines
    (split between them to balance), and stored contiguously.

f16 quantization of x/h/y adds ~3e-4 relative L2 error, far below the 2e-2
gate, and halves all HBM traffic vs fp32.  All FLOPs stay on the PE.

Sharding: pure data parallel - 64 signals / 8 cores = 8 signals per core.

Scheduling note: every TPB 64-byte instruction has a single semaphore-wait
slot, but Tile's slot-release deps routinely put 2+ waits on one
instruction (walrus then fails with "Too many sync wait commands").
_strip_redundant_waits post-processes the scheduled BIR: it computes
transitive completion guarantees (engine queues are in-order FIFO; an
instruction completes only after its waits held; a semaphore's v-th update
implies its earlier ones) and (a) drops waits provably implied by another
wait on the same instruction, (b) splits any remaining multi-wait set into
single-wait NoOps ahead of the instruction on the same queue.  The patched
BIR is returned via an instance-level to_json_bytes override that
bass2jax's lowering picks up.
"""

import math

import numpy as np

SAMPLE_RATE = 44100.0

# Problem geometry (hardcoded per harness contract).
B_FULL, C_FULL, T_FULL = 32, 2, 524288
N_CORES = 8
SIGS_PER_CORE = (B_FULL * C_FULL) // N_CORES  # 8
L = 128             # block size == PE array dim
NBLK = T_FULL // L   # 4096 block columns per signal
HALVES = 2           # pipeline units per signal (finer overlap)
HBLK = NBLK // HALVES  # block columns per half-signal
QW = 512             # matmul chunk width (1 PSUM bank of fp32)
NQ = HBLK // QW      # 4 chunks per half-signal
XP_ROW = 2176        # padded input row (halo + HBLK = 2049, 256B-aligned)


def _filter_coeffs(center_freq: float, q: float, gain: float):
    """torchaudio equalizer_biquad coefficients, normalized by a0 (float64)."""
    g = min(max(gain, 0.1), 10.0)
    w0 = 2.0 * math.pi * center_freq / SAMPLE_RATE
    A = math.exp(g / 40.0 * math.log(10.0))
    alpha = math.sin(w0) / (2.0 * q)
    b0 = 1.0 + alpha * A
    b1 = -2.0 * math.cos(w0)
    b2 = 1.0 - alpha * A
    a0 = 1.0 + alpha / A
    a1 = b1
    a2 = 1.0 - alpha / A
    return b0 / a0, b1 / a0, b2 / a0, a1 / a0, a2 / a0


def _impulse_response(center_freq: float, q: float, gain: float, n: int = 256):
    b0, b1, b2, a1, a2 = _filter_coeffs(center_freq, q, gain)
    h = np.zeros(n, dtype=np.float64)
    x1 = x2 = y1 = y2 = 0.0
    for i in range(n):
        xn = 1.0 if i == 0 else 0.0
        yn = b0 * xn + b1 * x1 + b2 * x2 - a1 * y1 - a2 * y2
        x2, x1 = x1, xn
        y2, y1 = y1, yn
        h[i] = yn
    return h


def _toeplitz_mats(h: np.ndarray):
    """T0[j,g] = h[g-j] (g>=j else 0); T1[j,g] = h[128+g-j].  Stationary
    matmul operands (lhsT): out = lhsT.T @ rhs."""
    j = np.arange(L)[:, None]
    g = np.arange(L)[None, :]
    d0 = g - j
    t0 = np.where(d0 >= 0, h[np.clip(d0, 0, len(h) - 1)], 0.0)
    d1 = 128 + g - j
    t1 = h[np.clip(d1, 0, len(h) - 1)]
    return t0.astype(np.float16), t1.astype(np.float16)


_NC_CACHE = {}


def _build_nc(n_sigs: int = SIGS_PER_CORE):
    """Build the per-core Bass program (same NEFF on all cores)."""
    import concourse.bass as bass
    import concourse.mybir as mybir
    import concourse.tile as tile

    f16 = mybir.dt.float16
    f32 = mybir.dt.float32
    nc = bass.Bass("TRN2")

    # Block-major input, one row per half-signal, with a halo column:
    # xp[u, j, 0] = X'[j, -1] of the half (zero for the first half, the
    # last column of the previous half otherwise), xp[u, j, 1+c] = X' col c.
    # Output block-major: yb[u, g, c].
    # Row stride padded to a multiple of 256B (2176 cols = 4352B) so every
    # partition line starts DRAM-page-aligned (dram-page-size=256).
    n_units = n_sigs * HALVES
    xp = nc.dram_tensor("xp", [n_units, L, XP_ROW], f16, kind="ExternalInput")
    t0d = nc.dram_tensor("t0", [L, L], f16, kind="ExternalInput")
    t1d = nc.dram_tensor("t1", [L, L], f16, kind="ExternalInput")
    yb = nc.dram_tensor("yb", [n_units, L, HBLK], f16, kind="ExternalOutput")

    with tile.TileContext(nc) as tc:
        with (
            tc.tile_pool(name="consts", bufs=1) as consts,
            tc.tile_pool(name="xs", bufs=8) as xs_pool,
            tc.tile_pool(name="yo", bufs=4) as yo_pool,
            tc.tile_pool(name="mm_ps", bufs=2, space="PSUM") as mm_ps,
        ):
            # Consts go over the (initially idle) store queue so the first
            # unit's load is not queued behind them.
            t0s = consts.tile([L, L], f16)
            t1s = consts.tile([L, L], f16)
            nc.scalar.dma_start(t0s[:], t0d[:])
            nc.scalar.dma_start(t1s[:], t1d[:])

            for u in range(n_units):
                # Loads dispatch from SP, stores from ACT: mixing directions
                # on one engine queue head-of-line-blocks later loads behind
                # a store dispatch that waits on compute.
                # ---- contiguous load (f16, 4KiB per partition line) ----
                xs = xs_pool.tile([L, HBLK + 1], f16)
                nc.sync.dma_start(xs[:], xp[u][:, : HBLK + 1])

                # ---- conv: all-T0 pass, then all-T1 pass; 4 PSUM banks per
                # unit, double-buffered across units ----
                pps = [
                    mm_ps.tile([L, QW], f32, tag=f"mm{q}", name=f"pp{q}")
                    for q in range(NQ)
                ]
                for q in range(NQ):
                    nc.tensor.matmul(
                        pps[q][:], t0s[:], xs[:, 1 + QW * q : 1 + QW * (q + 1)],
                        start=True, stop=False,
                    )
                for q in range(NQ):
                    nc.tensor.matmul(
                        pps[q][:], t1s[:], xs[:, QW * q : QW * (q + 1)],
                        start=False, stop=True,
                    )

                # ---- evac PSUM -> f16 (DVE 3 / ACT 1 per unit; GpSimd
                # cannot read PSUM), then store the unit ----
                yo = yo_pool.tile([L, HBLK], f16)
                evac_eng = [0, 1, 0, 0]  # 0=DVE, 1=ACT
                for q in range(NQ):
                    dst = yo[:, QW * q : QW * (q + 1)]
                    if evac_eng[q]:
                        nc.scalar.copy(dst, pps[q][:])
                    else:
                        nc.vector.tensor_copy(dst, pps[q][:])
                nc.scalar.dma_start(yb[u][:], yo[:])

    return nc


def _dedup_ldweights(bir_bytes: bytes) -> bytes:
    """Turn redundant PE Ldweights into NoOps.

    The emission interleaves `ld T0, mm, ld T0, mm, ...` within each all-T0
    (then all-T1) pass, but PE weights stay loaded across Matmults, so a
    Ldweights whose operand signature matches the previous one (with only
    Matmult/NoOp in between on the PE queue) is a pure reload.  walrus's
    ldw-opt would drop these but is compiled out (--enable-ldw-opt=false),
    so neutralize them here: opcode -> NoOp, operands cleared, sync_info
    (waits + semaphore updates) preserved so the schedule is unchanged."""
    import json

    bir = json.loads(bir_bytes)

    def signature(i):
        sig = {k: v for k, v in i.items()
               if k not in ("name", "sync_info", "debug")}
        return json.dumps(sig, sort_keys=True)

    n_dropped = 0

    def walk(block):
        nonlocal n_dropped
        last_sig = None
        for i in block.get("instructions", []):
            if i.get("engine") != "PE":
                continue
            op = i.get("opcode")
            if op == "Ldweights":
                sig = signature(i)
                if sig == last_sig:
                    keep = {"debug", "engine", "ins", "name", "opcode",
                            "outs", "sync_info"}
                    for k in [k for k in i if k not in keep]:
                        del i[k]
                    i["opcode"] = "NoOp"
                    i["ins"] = []
                    i["outs"] = []
                    n_dropped += 1
                else:
                    last_sig = sig
            elif op not in ("Matmult", "NoOp"):
                last_sig = None
        for sub in block.get("blocks", []):
            walk(sub)

    for b in bir["functions"][0]["blocks"]:
        walk(b)
    return json.dumps(bir).encode()


def _strip_redundant_waits(bir_bytes: bytes) -> bytes:
    """PE Matmult/Ldweights lower to TPB instructions with a single
    semaphore-wait slot, but Tile's slot-release deps put 2 waits (old-writer
    PE completion + old-reader DVE completion) on the first toucher of every
    reused PSUM slot.  The PE wait is transitively implied: the DVE evac copy
    whose completion the instruction also waits on had itself waited on those
    PE completions.  Prove the implication with a completion-guarantee
    dataflow (rules: an instruction completes only after its waits hold; TPB
    engine queues are in-order FIFO; a semaphore's v-th update implies its
    earlier updates) and drop provably-redundant waits; raise if a >1-wait
    matmul can't be reduced."""
    import json

    bir = json.loads(bir_bytes)
    insts = []
    containers = []  # (list, index) for each inst, for NoOp insertion

    def walk(block):
        lst = block.get("instructions", [])
        for idx, i in enumerate(lst):
            insts.append(i)
            containers.append((lst, idx))
        for sub in block.get("blocks", []):
            walk(sub)

    for b in bir["functions"][0]["blocks"]:
        walk(b)

    # Per-sem update timeline: list of (cumulative_value, inst_idx).
    timelines = {}
    for k, i in enumerate(insts):
        for u in i.get("sync_info", {}).get("on_update", []) or []:
            if u.get("sync_type") != "semaphore":
                continue
            tl = timelines.setdefault(u["ant_name"], [])
            prev = tl[-1][0] if tl else 0
            tl.append((prev + int(u.get("update_value", 1)), k))

    def producer(sem, val):
        """Index of the instruction whose update first brings sem >= val."""
        tl = timelines.get(sem)
        if not tl:
            return None
        import bisect
        pos = bisect.bisect_left(tl, (val, -1))
        if pos == len(tl):
            return None
        return tl[pos][1]

    IN_ORDER_ENGINES = {"PE", "DVE", "Activation", "Pool", "SP"}
    NOT_IN_ORDER_OPCODES = {"DMACopy", "DmaTransposeAnt"}  # complete on DMA queues

    # guarantees[k]: sem -> max value known to hold when inst k completes.
    guarantees = [dict() for _ in insts]
    prev_by_engine = {}
    preds = []  # per-inst: (same-engine pred, own waits, own updates)
    for k, i in enumerate(insts):
        eng = i.get("engine")
        in_order = eng in IN_ORDER_ENGINES and i.get("opcode") not in NOT_IN_ORDER_OPCODES
        pred = prev_by_engine.get(eng) if in_order else None
        preds.append(pred)
        if in_order:
            prev_by_engine[eng] = k

    def merge(dst, src):
        changed = False
        for s, v in src.items():
            if dst.get(s, 0) < v:
                dst[s] = v
                changed = True
        return changed

    for _pass in range(3):
        changed = False
        for k, i in enumerate(insts):
            g = guarantees[k]
            si = i.get("sync_info", {})
            for w in si.get("on_wait", []) or []:
                if w.get("sync_type") != "semaphore":
                    continue
                v = int(w["wait_value"])
                if g.get(w["ant_name"], 0) < v:
                    g[w["ant_name"]] = v
                    changed = True
                p = producer(w["ant_name"], v)
                if p is not None:
                    changed |= merge(g, guarantees[p])
            if preds[k] is not None:
                changed |= merge(g, guarantees[preds[k]])
        # Own updates fire at completion; same-sem update chains are FIFO
        # (engine queue or DMA queue), so the v-th updater inherits the
        # (v-1)-th updater's guarantees.
        for sem, tl in timelines.items():
            prev_idx = None
            for cum, k in tl:
                if guarantees[k].get(sem, 0) < cum:
                    guarantees[k][sem] = cum
                    changed = True
                if prev_idx is not None:
                    changed |= merge(guarantees[k], guarantees[prev_idx])
                prev_idx = k
        if not changed:
            break

    STRIP_OPCODES = {
        "Matmult", "Ldweights", "TensorCopy", "Memset", "DMACopy",
        "DmaTransposeAnt", "Activation", "TensorScalarAffineSelect",
        "TensorTensor", "TensorScalarPtr", "TensorReduce", "Drain", "NoOp",
    }
    stripped = 0
    inserts = []  # (list, index, [noop dicts])
    for k, i in enumerate(insts):
        if i.get("opcode") not in STRIP_OPCODES:
            continue
        si = i.get("sync_info", {})
        waits = si.get("on_wait", []) or []
        if len(waits) <= 1:
            continue
        # Drop every wait implied by another (not-yet-dropped) wait's
        # producer guarantee.
        kept = list(waits)
        changed = True
        while changed:
            changed = False
            for w in list(kept):
                if len(kept) == 1:
                    break
                for w2 in kept:
                    if w2 is w:
                        continue
                    p = producer(w2["ant_name"], int(w2["wait_value"]))
                    if p is not None and guarantees[p].get(w["ant_name"], 0) >= int(
                        w["wait_value"]
                    ):
                        kept.remove(w)
                        changed = True
                        break
        stripped += len(waits) - len(kept)
        si["on_wait"] = [kept[-1]]
        if len(kept) > 1:
            # Split remaining waits onto single-wait NoOps ahead of the
            # instruction on the same engine queue.
            lst, idx = containers[k]
            noops = [
                {
                    "debug": i.get("debug", 0),
                    "engine": i.get("engine"),
                    "ins": [],
                    "name": f"{i['name']}-w{j}",
                    "opcode": "NoOp",
                    "outs": [],
                    "sync_info": {"on_wait": [w], "on_update": []},
                }
                for j, w in enumerate(kept[:-1])
            ]
            inserts.append((lst, idx, noops))

    # Apply insertions (descending index per list keeps positions valid).
    from collections import defaultdict
    by_list = defaultdict(list)
    for lst, idx, noops in inserts:
        by_list[id(lst)].append((lst, idx, noops))
    for entries in by_list.values():
        for lst, idx, noops in sorted(entries, key=lambda e: -e[1]):
            lst[idx:idx] = noops

    out = json.dumps(bir).encode()
    return out


def audit_waits(bir_bytes):
    """Flag instructions with more than the single hardware wait slot."""
    import json

    bir = json.loads(bir_bytes)
    checked = {
        "Matmult", "Ldweights", "TensorCopy", "Memset", "DMACopy",
        "DmaTransposeAnt", "Activation", "TensorScalarAffineSelect",
        "TensorTensor", "TensorScalarPtr", "TensorReduce",
    }
    bad = []
    def walk(block):
        for i in block.get("instructions", []):
            if i.get("opcode") not in checked:
                continue
            w = i.get("sync_info", {}).get("on_wait", [])
            if len(w) > 1:
                bad.append((i["name"], i.get("opcode"), i.get("engine"),
                            [(x["ant_name"], x["wait_value"]) for x in w]))
        for sub in block.get("blocks", []):
            walk(sub)
    for b in bir["functions"][0]["blocks"]:
        walk(b)
    return bad


def _get_nc(n_sigs: int = SIGS_PER_CORE):
    # Note: _dedup_ldweights (NoOp-ing redundant weight reloads) measured
    # SLOWER on HW (59.0us vs 56.5us), so it is not applied.
    if n_sigs not in _NC_CACHE:
        nc = _build_nc(n_sigs)
        patched = _strip_redundant_waits(type(nc).to_json_bytes(nc))
        bad = audit_waits(patched)
        if bad:
            raise RuntimeError(f"multi-wait instructions remain: {bad[:5]}")
        nc.to_json_bytes = lambda: patched
        _NC_CACHE[n_sigs] = nc
    return _NC_CACHE[n_sigs]


def _to_blockmajor(x64: np.ndarray) -> np.ndarray:
    """[64, T] f16 -> [64*HALVES, 128, HBLK+1] f16 half-signal units.

    Unit (s, h) holds X'[s][:, 2048h : 2048(h+1)] in cols 1.., with col 0
    the halo (zero for h=0, else the last column of the previous half)."""
    n = x64.shape[0]
    xb = x64.reshape(n, NBLK, L).transpose(0, 2, 1)  # [n, 128, NBLK]
    xp = np.zeros((n, HALVES, L, XP_ROW), dtype=np.float16)
    for h_ in range(HALVES):
        xp[:, h_, :, 1 : HBLK + 1] = xb[:, :, HBLK * h_ : HBLK * (h_ + 1)]
        if h_ > 0:
            xp[:, h_, :, 0] = xb[:, :, HBLK * h_ - 1]
    return xp.reshape(n * HALVES, L, XP_ROW)


def _from_blockmajor(yb: np.ndarray) -> np.ndarray:
    """[64*HALVES, 128, HBLK] f16 -> [64, T] f16."""
    n = yb.shape[0] // HALVES
    yb = yb.reshape(n, HALVES, L, HBLK).transpose(0, 1, 3, 2)  # [n,H,HBLK,128]
    return np.ascontiguousarray(yb).reshape(n, T_FULL)


def run_spmd(x64: np.ndarray, t0: np.ndarray, t1: np.ndarray, trace: bool = False):
    """x64: [64, T] float16 -> [64, T] float16 (plus BassKernelResults)."""
    from concourse.bass_utils import run_bass_kernel_spmd

    nc = _get_nc()
    xp = _to_blockmajor(x64)
    upc = SIGS_PER_CORE * HALVES  # units per core
    in_maps = [
        {
            "xp": np.ascontiguousarray(xp[upc * c : upc * (c + 1)]),
            "t0": t0,
            "t1": t1,
        }
        for c in range(N_CORES)
    ]
    res = run_bass_kernel_spmd(
        nc, in_maps, core_ids=list(range(N_CORES)), trace=trace
    )
    yb = np.concatenate([res.results[c]["yb"] for c in range(N_CORES)], axis=0)
    return _from_blockmajor(yb), res


def kernel(x, center_freq, q, gain, t=0, **_unused):
    x = np.asarray(x)
    assert x.shape == (B_FULL, C_FULL, T_FULL), x.shape
    cf = float(np.asarray(center_freq).reshape(-1)[0])
    qv = float(np.asarray(q).reshape(-1)[0])
    gv = float(np.asarray(gain).reshape(-1)[0])

    h = _impulse_response(cf, qv, gv)
    t0, t1 = _toeplitz_mats(h)

    x64 = np.ascontiguousarray(
        x.reshape(B_FULL * C_FULL, T_FULL), dtype=np.float16
    )
    out, _ = run_spmd(x64, t0, t1, trace=False)
    return out.reshape(B_FULL, C_FULL, T_FULL).astype(np.float32)
